# revision 29
# baseline (speedup 1.0000x reference)
"""Cross-attention kernel for Trainium2, data-parallel over batch on 8 NeuronCores.

Reference computation (per batch element b):
    lat = LN(latent_q[b]) ; inp = LN(input_kv[b])
    Q = lat @ W_Q ; K = inp @ W_K ; V = inp @ W_V      (8 heads x 128 dims)
    out[b] = softmax(Q K^T / sqrt(128)) V @ W_O

Sharding: batch B=8 -> one batch element per core, zero collectives.

v4 design (from v3 + sim-trace analysis):
  - PSUM banks repartitioned so no matmul ever waits on a PSUM drain:
    S 2x[128,512] (bufs=2), K 1x[128,2,512], V 1x[128,512] (bufs=1,
    half-drains), O 1x[128,2,512], l 1x[128,512] persistent. Every bank's
    reuse distance is >= ~6 matmuls ahead of its drain.
  - softmax denominator l accumulated ON THE PE: per pT tile one extra
    matmul with a one-hot [128,8] stationary E_h into a persistent PSUM
    bank (start only on the first, stop on the last). Replaces ~525us of
    DVE/Pool tensor_adds per rep and the whole epilogue partition-reduce.
  - rsqrt for LN computed with DVE reciprocal + 3 Newton steps on tiny
    [128,4] tiles: removes the Ln activation -> no act-table thrash
    (v3 paid 2x1283ns LoadActFuncSet per chunk); exp is the only
    table-dependent ACT function left.
  - emission order interleaves S into the K/V accumulation chains so exp
    results are always ready ~>=6 MMs before their S bank is reused, and
    V half-drains overlap the O matmuls.
  - weight DMAs/constants hoisted out of the reps loop (For_i) so the
    rep-slope timing measures steady state; o_acc uses copy-on-first-chunk
    instead of a memset (no cross-rep serialization).
  - kv-chunk pipeline as v3: x DMA 2 chunks ahead (gpsimd queue); LN
    stats/apply/DRAM-bounce/transpose chain for chunk c+1 emitted in the
    middle of chunk c (after hp==1).
"""

import numpy as np
import ml_dtypes

import concourse.bass as bass
import concourse.mybir as mybir
import concourse.tile as tile
from concourse import bacc
from concourse.bass_utils import run_bass_kernel_spmd

AF = mybir.ActivationFunctionType
DT = mybir.dt
ALU = mybir.AluOpType

B = 8
LQ = 512
LKV = 16384
DLAT = 1024
DIN = 768
QK_CH = 1024
V_CH = 1024
OUT_CH = 1024
H = 8
DH = 128
P = 128
EPS = 1e-5
SCALE = float(1.0 / np.sqrt(DH))

CHUNK = 512               # kv rows per chunk
N_KV_T = CHUNK // P       # 4
N_LQ_T = LQ // P          # 4
N_LAT_S = DLAT // P       # 8
N_IN_S = DIN // P         # 6
N_VC_S = V_CH // P        # 8


def build_program(lkv=LKV, reps=1, with_biases=False, skip=(),
                  l_dve_heads=2, l_pool_heads=2, kt_dve=False, v1_act=False,
                  sbuf_t=False):
    # skip: subset of {"exp","l","drain","stage","attn"} for differential
    # profiling (numerics intentionally wrong when nonempty)
    # l_dve_heads / l_pool_heads: move l accumulation for that many of the
    #   last heads from PE matmuls to DVE / Pool tensor_adds (rebalance knobs)
    # kt_dve: drain the K projection PSUM on DVE instead of ACT
    n_chunks = lkv // CHUNK
    l_off = l_dve_heads + l_pool_heads
    n_lpe = H - l_off            # heads whose l stays on the PE

    nc = bacc.Bacc()
    lq_d = nc.dram_tensor("lq", [LQ, DLAT], DT.float32, kind="ExternalInput")
    xkv_d = nc.dram_tensor("xkv", [lkv, DIN], DT.float32, kind="ExternalInput")
    wq_d = nc.dram_tensor("wq", [DLAT, QK_CH], DT.bfloat16, kind="ExternalInput")
    wk_d = nc.dram_tensor("wk", [DIN, QK_CH], DT.bfloat16, kind="ExternalInput")
    wv_d = nc.dram_tensor("wv", [DIN, V_CH], DT.bfloat16, kind="ExternalInput")
    wo_d = nc.dram_tensor("wo", [V_CH, OUT_CH], DT.bfloat16, kind="ExternalInput")
    if with_biases:
        tq_d = nc.dram_tensor("tq", [P, H], DT.float32, kind="ExternalInput")
        tk_d = nc.dram_tensor("tk", [P, H], DT.float32, kind="ExternalInput")
        tvb_d = nc.dram_tensor("tvb", [P, V_CH], DT.bfloat16, kind="ExternalInput")
    eb_d = nc.dram_tensor("eb", [H, H * P], DT.bfloat16, kind="ExternalInput")
    out_d = nc.dram_tensor("out", [LQ, OUT_CH], DT.float32, kind="ExternalOutput")

    with tile.TileContext(nc) as tc:
        with (
            tc.tile_pool(name="weights", bufs=1) as wpool,
            tc.tile_pool(name="persist", bufs=1) as perpool,
            tc.tile_pool(name="xin", bufs=2) as xpool,
            tc.tile_pool(name="xn", bufs=2) as xnpool,
            tc.tile_pool(name="xnt", bufs=2) as xntpool,
            tc.tile_pool(name="kt", bufs=2) as ktpool,
            tc.tile_pool(name="vt", bufs=2) as vpool,
            tc.tile_pool(name="pt", bufs=8) as ptpool,
            tc.tile_pool(name="stats", bufs=3) as stats_pool,
            tc.tile_pool(name="dram", bufs=3, space="DRAM") as dram_pool,
            tc.tile_pool(name="spsum", bufs=2, space="PSUM") as spsum,
            tc.tile_pool(name="kpsum", bufs=1, space="PSUM") as kpsum,
            tc.tile_pool(name="vpsum", bufs=1, space="PSUM") as vpsum,
            tc.tile_pool(name="opsum", bufs=1, space="PSUM") as opsum,
            tc.tile_pool(name="lpsum", bufs=1, space="PSUM") as lpsum,
        ):
            # ---- weight/constant tiles ----
            wq_sb = wpool.tile([P, N_LAT_S, QK_CH], DT.bfloat16)
            wk_sb = wpool.tile([P, N_IN_S, QK_CH], DT.bfloat16)
            wv_sb = wpool.tile([P, N_IN_S, V_CH], DT.bfloat16)
            wo_sb = wpool.tile([P, N_VC_S, OUT_CH], DT.bfloat16)
            if with_biases:
                tq_sb = wpool.tile([P, H], DT.float32)
                tk_sb = wpool.tile([P, H], DT.float32)
                tvb_sb = wpool.tile([P, 2, 512], DT.bfloat16)
            e_sb = wpool.tile([P, H, H], DT.bfloat16)   # E_h one-hot columns
            eb_sb = wpool.tile([P, H, P], DT.bfloat16)  # A_h one-hot rows (bcast)
            if l_off:
                e32_sb = wpool.tile([P, l_off, H], DT.float32)

            q_sb = perpool.tile([P, H, LQ], DT.bfloat16)
            o_acc = perpool.tile([P, H, LQ], DT.float32)
            if l_off:
                l_acc = perpool.tile([P, l_off, LQ], DT.float32)

            def setup():
                nc.scalar.dma_start(wk_sb[:], wk_d[:].rearrange("(s p) n -> p s n", p=P))
                nc.scalar.dma_start(wv_sb[:], wv_d[:].rearrange("(s p) n -> p s n", p=P))
                nc.scalar.dma_start(wq_sb[:], wq_d[:].rearrange("(s p) n -> p s n", p=P))
                nc.scalar.dma_start(wo_sb[:], wo_d[:].rearrange("(s p) n -> p s n", p=P))
                if with_biases:
                    nc.scalar.dma_start(tq_sb[:], tq_d[:])
                    nc.gpsimd.dma_start(tk_sb[:], tk_d[:])
                    nc.scalar.dma_start(tvb_sb[:], tvb_d[:])
                nc.gpsimd.memset(e_sb[:], 0.0)
                for h in range(H):
                    nc.gpsimd.memset(e_sb[:, h, h : h + 1], 1.0)
                if l_off:
                    nc.gpsimd.memset(e32_sb[:], 0.0)
                    for i in range(l_off):
                        h = n_lpe + i
                        nc.gpsimd.memset(e32_sb[:, i, h : h + 1], 1.0)
                nc.gpsimd.dma_start(
                    eb_sb[0:H, :, :], eb_d[:].rearrange("p (h n) -> p h n", h=H)
                )

            def ln_stats(x_ap, n_sub, width):
                """LN stats for [128, n_sub, width] fp32 -> (inv, nmi) [128, n_sub].

                inv = rsqrt(var+eps) via DVE reciprocal seed + 3 Newton steps
                (no ACT Ln/Exp -> no act-table switches). nmi = -mean*inv.
                """
                half = width // 2
                st = stats_pool.tile([P, n_sub, 12], DT.float32, tag="bnst")
                mv = stats_pool.tile([P, n_sub, 2], DT.float32, tag="bnmv")
                for t in range(n_sub):
                    nc.vector.bn_stats(st[:, t, 0:6], x_ap[:, t, 0:half])
                    nc.vector.bn_stats(st[:, t, 6:12], x_ap[:, t, half:width])
                    nc.vector.bn_aggr(mv[:, t, :], st[:, t, :])
                v = stats_pool.tile([P, n_sub], DT.float32, tag="bnv")
                nc.vector.tensor_scalar(v[:], mv[:, :, 1], 1.0, EPS, ALU.mult, ALU.add)
                inv = stats_pool.tile([P, n_sub], DT.float32, tag="bninv")
                nc.vector.reciprocal(inv[:], v[:])  # seed y0 = 1/v
                a = stats_pool.tile([P, n_sub], DT.float32, tag="bna")
                tt = stats_pool.tile([P, n_sub], DT.float32, tag="bnt")
                for _ in range(3):  # y <- y*(1.5 - 0.5*v*y^2)
                    nc.vector.tensor_mul(a[:], inv[:], inv[:])
                    nc.vector.tensor_mul(a[:], a[:], v[:])
                    nc.vector.tensor_scalar(tt[:], a[:], -0.5, 1.5, ALU.mult, ALU.add)
                    nc.vector.tensor_mul(inv[:], inv[:], tt[:])
                nmi = stats_pool.tile([P, n_sub], DT.float32, tag="bnnmi")
                nc.vector.tensor_mul(nmi[:], mv[:, :, 0], inv[:])
                nc.vector.tensor_scalar_mul(nmi[:], nmi[:], -1.0)
                return inv, nmi

            def body():
                xkv_r = xkv_d[:].rearrange("(c t p) ch -> c p t ch", t=N_KV_T, p=P)

                def stage_dma(c):
                    """Kick the x DMA for chunk c (2 chunks ahead)."""
                    x_t = xpool.tile([P, N_KV_T, DIN], DT.float32, tag="x")
                    nc.gpsimd.dma_start(x_t[:], xkv_r[c])
                    return x_t

                def stage_ln(x_t):
                    """LN + bf16 + DRAM bounce + transpose for a loaded chunk."""
                    inv, nmi = ln_stats(x_t, N_KV_T, DIN)
                    xn_t = xnpool.tile([P, N_KV_T, DIN], DT.bfloat16, tag="xn")
                    for t in range(N_KV_T):
                        nc.gpsimd.tensor_scalar(
                            xn_t[:, t, :], x_t[:, t, :],
                            inv[:, t : t + 1], nmi[:, t : t + 1], ALU.mult, ALU.add,
                        )
                    xnT = xntpool.tile([P, N_IN_S, CHUNK], DT.bfloat16)
                    if sbuf_t:
                        for t in range(N_KV_T):
                            nc.sync.dma_start_transpose(
                                xnT[:, :, t * P : (t + 1) * P], xn_t[:, t, :]
                            )
                    else:
                        xnd = dram_pool.tile([CHUNK, DIN], DT.bfloat16, tag="xnd")
                        nc.sync.dma_start(
                            xnd[:].rearrange("(t p) ch -> p t ch", p=P), xn_t[:]
                        )
                        nc.sync.dma_start_transpose(xnT[:], xnd[:])
                    return xnT

                # prime the pipeline: x(0), x(1) DMAs; LN(0)
                x_pend = stage_dma(0)
                if n_chunks > 1 and "stage" not in skip:
                    x_next = stage_dma(1)
                xnT_cur = stage_ln(x_pend)
                x_pend = x_next if (n_chunks > 1 and "stage" not in skip) else None

                # ---------- latent LN -> DMA transpose (DMA/DVE/Pool side) ----
                # Emitted before the chunk loop so the chain's latency hides
                # behind chunk 0's K/V matmuls; the Q-projection matmuls are
                # emitted after chunk 0 (emit_qproj below).
                latnT = ktpool.tile([P, N_LAT_S, LQ], DT.bfloat16, tag="kT")
                if not sbuf_t:
                    latd = dram_pool.tile([LQ, DLAT], DT.bfloat16, tag="latd")
                    latd_r = latd[:].rearrange("(t p) n -> t p n", p=P)
                lq_r = lq_d[:].rearrange("(t p) n -> t p n", p=P)
                for t in range(N_LQ_T):
                    lat_t = xpool.tile([P, 1, DLAT], DT.float32, tag="lat")
                    nc.sync.dma_start(lat_t[:, 0, :], lq_r[t])
                    inv, nmi = ln_stats(lat_t, 1, DLAT)
                    latn = xnpool.tile([P, DLAT], DT.bfloat16, tag="xn")
                    nc.gpsimd.tensor_scalar(
                        latn[:], lat_t[:, 0, :],
                        inv[:, 0:1], nmi[:, 0:1], ALU.mult, ALU.add,
                    )
                    if sbuf_t:
                        nc.sync.dma_start_transpose(
                            latnT[:, :, t * P : (t + 1) * P], latn[:]
                        )
                    else:
                        nc.sync.dma_start(latd_r[t], latn[:])
                if not sbuf_t:
                    nc.sync.dma_start_transpose(latnT[:], latd[:])

                def emit_qproj():
                    for h in range(H):
                        qps = spsum.tile([P, LQ], DT.float32, tag="s")
                        for s in range(N_LAT_S):
                            nc.tensor.matmul(
                                qps[:],
                                wq_sb[:, s, h * DH : (h + 1) * DH],
                                latnT[:, s, :],
                                start=(s == 0),
                                stop=(s == N_LAT_S - 1),
                            )
                        if with_biases:
                            nc.vector.tensor_scalar(
                                q_sb[:, h, :], qps[:],
                                tq_sb[:, h : h + 1], None, ALU.add,
                            )
                        else:
                            nc.vector.tensor_copy(q_sb[:, h, :], qps[:])

                l_ps = lpsum.tile([P, LQ], DT.float32)
                n_att = 0          # attention chunks processed so far
                n_att_total = n_chunks if "attn" not in skip else 0

                def emit_s1(kT, h, t):
                    """One S matmul [128kv,512q] + exp -> pT (single-bank)."""
                    sps = spsum.tile([P, LQ], DT.float32, tag="s")
                    nc.tensor.matmul(
                        sps[:],
                        kT[:, h, t * P : (t + 1) * P],
                        q_sb[:, h, :],
                        start=True,
                        stop=True,
                    )
                    pT = ptpool.tile([P, LQ], DT.bfloat16)
                    if "exp" not in skip:
                        nc.scalar.activation(pT[:], sps[:], AF.Exp, scale=SCALE)
                    return pT

                def emit_o(v_t, h, half, pts, first_att, last_att):
                    """O matmuls (+ l matmuls/adds) for head h of the previous chunk."""
                    for t in range(N_KV_T):
                        nc.tensor.matmul(
                            ops2[:, half, :],
                            v_t[:, t, h // 4, (h % 4) * DH : (h % 4 + 1) * DH],
                            pts[t][:],
                            start=(t == 0),
                            stop=(t == N_KV_T - 1),
                        )
                        if "l" in skip:
                            continue
                        if h < n_lpe:
                            nc.tensor.matmul(
                                l_ps[0:H, :],
                                e_sb[:, h, :],
                                pts[t][:],
                                start=(first_att and h == 0 and t == 0),
                                stop=(not l_off and last_att
                                      and h == H - 1 and t == N_KV_T - 1),
                            )
                        else:
                            eng = (
                                nc.vector
                                if h < n_lpe + l_dve_heads
                                else nc.gpsimd
                            )
                            if first_att and t == 0:
                                eng.tensor_copy(l_acc[:, h - n_lpe, :], pts[t][:])
                            else:
                                eng.tensor_add(
                                    l_acc[:, h - n_lpe, :],
                                    l_acc[:, h - n_lpe, :],
                                    pts[t][:],
                                )

                def emit_k(xnT, h, half, s0, s1):
                    """K projection slices [s0,s1) for head h into kpsum half."""
                    for s in range(s0, s1):
                        nc.tensor.matmul(
                            kps2[:, half, :],
                            wk_sb[:, s, h * DH : (h + 1) * DH],
                            xnT[:, s, :],
                            start=(s == 0),
                            stop=(s == N_IN_S - 1),
                        )

                def emit_v(xnT, tt, nf, s0, s1):
                    """V projection slices [s0,s1) for kv-subtile tt, dv half nf."""
                    for s in range(s0, s1):
                        nc.tensor.matmul(
                            vps[:],
                            xnT[:, s, tt * P : (tt + 1) * P],
                            wv_sb[:, s, nf * 512 : (nf + 1) * 512],
                            start=(s == 0),
                            stop=(s == N_IN_S - 1),
                        )

                prev = None
                for c in range(n_chunks):
                    xnT = xnT_cur
                    xnT_next = None
                    kT = ktpool.tile([P, H, CHUNK], DT.bfloat16, tag="kT")
                    v_t = vpool.tile([P, N_KV_T, 2, 512], DT.bfloat16)
                    first_att = prev is not None and n_att == 0
                    last_att = prev is not None and (n_att + 1 == n_att_total)
                    for hp in range(H // 2):
                        h0, h1 = 2 * hp, 2 * hp + 1
                        att = prev is not None and "attn" not in skip
                        pts0, pts1 = [], []
                        # --- K(h0) with S(h0) interleaved ---
                        kps2 = kpsum.tile([P, 2, CHUNK], DT.float32, tag="k")
                        if att:
                            pts0.append(emit_s1(prev[0], h0, 0))
                        emit_k(xnT, h0, 0, 0, 2)
                        if att:
                            pts0.append(emit_s1(prev[0], h0, 1))
                        emit_k(xnT, h0, 0, 2, 4)
                        if att:
                            pts0.append(emit_s1(prev[0], h0, 2))
                        emit_k(xnT, h0, 0, 4, 6)
                        if att:
                            pts0.append(emit_s1(prev[0], h0, 3))
                        # --- K(h1) with S(h1,0..1) interleaved ---
                        emit_k(xnT, h1, 1, 0, 2)
                        if att:
                            pts1.append(emit_s1(prev[0], h1, 0))
                        emit_k(xnT, h1, 1, 2, 4)
                        if att:
                            pts1.append(emit_s1(prev[0], h1, 1))
                        emit_k(xnT, h1, 1, 4, 6)
                        if with_biases:
                            for hh, half in ((h0, 0), (h1, 1)):
                                nc.vector.tensor_scalar(
                                    kT[:, hh, :], kps2[:, half, :],
                                    tk_sb[:, hh : hh + 1], None, ALU.add,
                                )
                        elif "drain" not in skip:
                            if kt_dve:
                                nc.vector.tensor_copy(kT[:, h0 : h0 + 2, :], kps2[:])
                            else:
                                nc.scalar.copy(kT[:, h0 : h0 + 2, :], kps2[:])
                        # --- V half 0 ---
                        vps = vpsum.tile([P, 512], DT.float32, tag="v")
                        emit_v(xnT, hp, 0, 0, 6)
                        if with_biases:
                            nc.vector.tensor_add(
                                v_t[:, hp, 0, :], vps[:], tvb_sb[:, 0, :]
                            )
                        elif "drain" not in skip:
                            nc.scalar.copy(v_t[:, hp, 0, :], vps[:])
                        # weave next chunk's LN chain here (overlaps hp 1-3)
                        if hp == 1 and "stage" not in skip:
                            if c + 2 < n_chunks:
                                x_new = stage_dma(c + 2)
                            if x_pend is not None:
                                xnT_next = stage_ln(x_pend)
                        # --- remaining S(h1) ---
                        if att:
                            pts1.append(emit_s1(prev[0], h1, 2))
                            pts1.append(emit_s1(prev[0], h1, 3))
                        # --- O(h0) + l(h0) ---
                        ops2 = opsum.tile([P, 2, LQ], DT.float32, tag="o")
                        if att:
                            emit_o(prev[1], h0, 0, pts0, first_att, last_att)
                        # --- V half 1 ---
                        vps = vpsum.tile([P, 512], DT.float32, tag="v")
                        emit_v(xnT, hp, 1, 0, 6)
                        if with_biases:
                            nc.vector.tensor_add(
                                v_t[:, hp, 1, :], vps[:], tvb_sb[:, 1, :]
                            )
                        elif "drain" not in skip:
                            if v1_act:
                                nc.scalar.copy(v_t[:, hp, 1, :], vps[:])
                            else:
                                nc.vector.tensor_copy(v_t[:, hp, 1, :], vps[:])
                        # --- O(h1) + l(h1) ---
                        if att:
                            emit_o(prev[1], h1, 1, pts1, first_att, last_att)
                            if "drain" not in skip:
                                if n_att == 0:
                                    nc.vector.tensor_copy(
                                        o_acc[:, h0 : h0 + 2, :], ops2[:]
                                    )
                                else:
                                    nc.vector.tensor_add(
                                        o_acc[:, h0 : h0 + 2, :],
                                        o_acc[:, h0 : h0 + 2, :],
                                        ops2[:],
                                    )
                    if c == 0:
                        emit_qproj()
                    if prev is not None:
                        n_att += 1
                    if c + 2 < n_chunks and "stage" not in skip:
                        x_pend = x_new
                    else:
                        x_pend = None
                    xnT_cur = xnT_next if "stage" not in skip else xnT
                    prev = (kT, v_t)

                # flush: attention for the last chunk
                if "attn" not in skip:
                    first_att = n_att == 0
                    last_att = n_att + 1 == n_att_total
                    for hp in range(H // 2):
                        h0, h1 = 2 * hp, 2 * hp + 1
                        pts0 = []
                        pts0.append(emit_s1(prev[0], h0, 0))
                        pts0.append(emit_s1(prev[0], h0, 1))
                        pts0.append(emit_s1(prev[0], h0, 2))
                        pts0.append(emit_s1(prev[0], h0, 3))
                        pts1 = []
                        pts1.append(emit_s1(prev[0], h1, 0))
                        pts1.append(emit_s1(prev[0], h1, 1))
                        ops2 = opsum.tile([P, 2, LQ], DT.float32, tag="o")
                        emit_o(prev[1], h0, 0, pts0, first_att, last_att)
                        pts1.append(emit_s1(prev[0], h1, 2))
                        pts1.append(emit_s1(prev[0], h1, 3))
                        emit_o(prev[1], h1, 1, pts1, first_att, last_att)
                        if "drain" not in skip:
                            if n_att == 0:
                                nc.vector.tensor_copy(
                                    o_acc[:, h0 : h0 + 2, :], ops2[:]
                                )
                            else:
                                nc.vector.tensor_add(
                                    o_acc[:, h0 : h0 + 2, :],
                                    o_acc[:, h0 : h0 + 2, :],
                                    ops2[:],
                                )
                    n_att += 1

                # ---------- epilogue: 1/l, normalize, project with W_O ----------
                lf_sb = stats_pool.tile([P, LQ], DT.float32, tag="lsb")
                l_sb = stats_pool.tile([P, LQ], DT.bfloat16, tag="lsb16")
                if "l" not in skip and "attn" not in skip:
                    for i in range(l_off):
                        nc.tensor.matmul(
                            l_ps[0:H, :],
                            e32_sb[:, i, :],
                            l_acc[:, i, :],
                            start=False,
                            stop=(i == l_off - 1),
                        )
                    nc.scalar.copy(lf_sb[0:H, :], l_ps[0:H, :])
                    nc.vector.reciprocal(lf_sb[0:H, :], lf_sb[0:H, :])
                    nc.vector.tensor_copy(l_sb[0:H, :], lf_sb[0:H, :])
                else:
                    nc.gpsimd.memset(l_sb[0:H, :], 1.0)
                o_n = ktpool.tile([P, H, LQ], DT.bfloat16, tag="kT")
                for h in range(H):
                    bps = spsum.tile([P, LQ], DT.float32, tag="s")
                    nc.tensor.matmul(
                        bps[:], eb_sb[0:H, h, :], l_sb[0:H, :],
                        start=True, stop=True,
                    )
                    nc.vector.tensor_mul(o_n[:, h, :], o_acc[:, h, :], bps[:])
                for nf in range(OUT_CH // 512):
                    out_sb = xnpool.tile([P, N_LQ_T, 512], DT.float32, tag="xn")
                    for qt in range(N_LQ_T):
                        octile = spsum.tile([P, 512], DT.float32, tag="s")
                        for s in range(N_VC_S):
                            nc.tensor.matmul(
                                octile[:],
                                o_n[:, s, qt * P : (qt + 1) * P],
                                wo_sb[:, s, nf * 512 : (nf + 1) * 512],
                                start=(s == 0),
                                stop=(s == N_VC_S - 1),
                            )
                        if nf == 0 and qt < 2:
                            nc.scalar.copy(out_sb[:, qt, :], octile[:])
                        else:
                            nc.vector.tensor_copy(out_sb[:, qt, :], octile[:])
                    nc.sync.dma_start(
                        out_d[:].rearrange("(t p) n -> p t n", p=P)[
                            :, :, nf * 512 : (nf + 1) * 512
                        ],
                        out_sb[:],
                    )

            setup()
            if reps == 1:
                body()
            else:
                with tc.For_i(0, reps, 1) as _i:
                    body()

    nc.compile()
    return nc


def host_prep(W_Q, W_K, W_V, W_O, ln_lat_g, ln_lat_b, ln_in_g, ln_in_b):
    """Fold LN affine params into weights; returns device input dict pieces.
    Bias terms (from LN beta) are included only when nonzero."""
    bf16 = ml_dtypes.bfloat16
    wq = (ln_lat_g[:, None].astype(np.float64) * W_Q.astype(np.float64)).astype(bf16)
    wk = (ln_in_g[:, None].astype(np.float64) * W_K.astype(np.float64)).astype(bf16)
    wv = (ln_in_g[:, None].astype(np.float64) * W_V.astype(np.float64)).astype(bf16)
    wo = W_O.astype(bf16)
    eb = np.zeros((H, H, P), dtype=bf16)
    for h in range(H):
        eb[h, h, :] = 1
    res = dict(wq=wq, wk=wk, wv=wv, wo=wo, eb=eb.reshape(H, H * P))
    if np.any(ln_lat_b != 0) or np.any(ln_in_b != 0):
        tq = (ln_lat_b.astype(np.float64) @ W_Q.astype(np.float64)).astype(np.float32)
        tk = (ln_in_b.astype(np.float64) @ W_K.astype(np.float64)).astype(np.float32)
        tv = (ln_in_b.astype(np.float64) @ W_V.astype(np.float64)).astype(np.float32)
        res["tq"] = np.ascontiguousarray(tq.reshape(H, DH).T)
        res["tk"] = np.ascontiguousarray(tk.reshape(H, DH).T)
        res["tvb"] = np.ascontiguousarray(np.broadcast_to(tv.astype(bf16), (P, V_CH)))
    return res


_prog_cache = {}


def _get_program(with_biases):
    key = ("main", with_biases)
    if key not in _prog_cache:
        _prog_cache[key] = build_program(with_biases=with_biases)
    return _prog_cache[key]


def kernel(latent_q, input_kv, W_Q, W_K, W_V, W_O,
           ln_lat_g, ln_lat_b, ln_in_g, ln_in_b):
    shared = host_prep(W_Q, W_K, W_V, W_O, ln_lat_g, ln_lat_b, ln_in_g, ln_in_b)
    nc = _get_program("tq" in shared)
    in_maps = [
        dict(
            lq=np.ascontiguousarray(latent_q[b]),
            xkv=np.ascontiguousarray(input_kv[b]),
            **shared,
        )
        for b in range(B)
    ]
    res = run_bass_kernel_spmd(nc, in_maps, list(range(B)))
    out = np.stack([res.results[b]["out"] for b in range(B)])
    return out.astype(np.float32)
